# revision 18
# baseline (speedup 1.0000x reference)
import sys

import numpy as np

sys.path.insert(0, "/opt/trn_rl_repo")

from concourse import bacc, bass, tile  # noqa: E402,F401
from concourse import mybir  # noqa: E402
from concourse.bass import broadcast_tensor_aps  # noqa: E402
from concourse.bass_utils import run_bass_kernel_spmd  # noqa: E402

N_CORES = 8
S = 8  # samples per core
C = 3
T = 9
H = W = 256
RC = 4  # rows per chunk (one SBUF partition holds one chunk)
NCH = H // RC  # 64 chunks per sample
RP = RC + 2  # row slots incl top/bottom halo
WP = W + 2  # col slots incl left/right reflect pad
F32 = mybir.dt.float32
F16 = mybir.dt.float16
NPROD = 2  # product ring depth (DVE-only ring: program order covers WAR)


def build_nc():
    nc = bacc.Bacc()
    x_ext = nc.declare_dram_parameter("x", [S, C, H, W], F16, isOutput=False)
    sg_ext = nc.declare_dram_parameter("sigma", [S, T, H, W], F16, isOutput=False)
    out_ext = nc.declare_dram_parameter("out", [S, C, H, W], F32, isOutput=True)

    with tile.TileContext(nc) as tc:
        # gpool has one buffer set per stripe -> gp-touched tiles are never
        # recycled (gpsimd + pool recycle faults HW with NRT 101)
        with tc.tile_pool(name="p", bufs=2) as pool, tc.tile_pool(
            name="g", bufs=S // 2
        ) as gpool:
            for stripe in range(S // 2):
                xt = pool.tile([128, C, RP, WP], F16)
                st = pool.tile([128, T, RC, W], F16)
                prods = [
                    pool.tile([128, C, RC, W], F16, name=f"prod{j}")
                    for j in range(NPROD)
                ]
                acc = gpool.tile([128, C, RC, W], F16)
                den16 = pool.tile([128, RC, W], F16)
                den = pool.tile([128, 1, RC, W], F32)
                inv = gpool.tile([128, 1, RC, W], F32)
                ot = gpool.tile([128, C, RC, W], F32)

                for k in range(2):
                    s = 2 * stripe + k
                    pb = 64 * k
                    # disjoint partition halves -> run the two samples' DMAs
                    # on separate engine queues
                    eng = nc.sync if k == 0 else nc.scalar
                    xr = x_ext[s].rearrange("c (n r) w -> n c r w", r=RC)
                    sr = sg_ext[s].rearrange("t (n r) w -> n t r w", r=RC)
                    # main rows -> slots 1..RC, image cols -> slots 1..W
                    # (DMA APs are limited to 3 dims -> one DMA per channel)
                    for c in range(C):
                        eng.dma_start(
                            xt[pb : pb + 64, c, 1 : 1 + RC, 1 : 1 + W], xr[:, c]
                        )
                    # top halo row: chunks 1..63 read prev chunk row 3
                    eng.dma_start(
                        xt[pb + 1 : pb + 64, :, 0, 1 : 1 + W], xr[0:63, :, 3, :]
                    )
                    # chunk 0 top halo: reflect row 1
                    eng.dma_start(xt[pb : pb + 1, :, 0, 1 : 1 + W], xr[0:1, :, 1, :])
                    # bottom halo row: chunks 0..62 read next chunk row 0
                    eng.dma_start(xt[pb : pb + 63, :, 5, 1 : 1 + W], xr[1:64, :, 0, :])
                    # chunk 63 bottom halo: reflect row 254 (= chunk 63 row 2)
                    eng.dma_start(
                        xt[pb + 63 : pb + 64, :, 5, 1 : 1 + W], xr[63:64, :, 2, :]
                    )
                    eng.dma_start(st[pb : pb + 64], sr)

                # column reflect pads: slot 0 <- image col 1 (slot 2),
                # slot WP-1 <- image col W-2 (slot WP-3)
                nc.scalar.copy(xt[:, :, :, 0:1], xt[:, :, :, 2:3])
                nc.scalar.copy(xt[:, :, :, WP - 1 : WP], xt[:, :, :, WP - 3 : WP - 2])

                # All compute on DVE: gpsimd touching recycled pool buffers
                # faults HW (NRT_EXEC_UNIT_UNRECOVERABLE 101). fp16 keeps
                # DVE in 2x_1p perf mode.
                with nc.allow_low_precision(reason="fp16 kernel"):
                    for t in range(T):
                        di, dj = t // 3, t % 3
                        xs = xt[:, :, di : di + RC, dj : dj + W]
                        sg = st[:, t : t + 1]
                        a, b = broadcast_tensor_aps(xs, sg)
                        nc.vector.tensor_mul(prods[t % NPROD][:], a, b)
                        if t == 1:
                            nc.vector.tensor_add(acc[:], prods[0][:], prods[1][:])
                        elif t > 1:
                            nc.vector.tensor_add(
                                acc[:], acc[:], prods[t % NPROD][:]
                            )

                    # DVE: denominator = sum sigma over taps (fp16 chain,
                    # fp32 tail), then reciprocal
                    nc.vector.tensor_add(den16[:], st[:, 0], st[:, 1])
                    for t in range(2, T - 1):
                        nc.vector.tensor_add(den16[:], den16[:], st[:, t])
                nc.vector.tensor_add(den[:, 0], den16[:], st[:, T - 1])
                nc.vector.reciprocal(inv[:], den[:])

                # normalize on gpsimd (fp32 out) — frees ~3.7us/stripe of DVE
                a, b = broadcast_tensor_aps(acc[:], inv[:])
                nc.gpsimd.tensor_mul(ot[:], a, b)

                for k in range(2):
                    s = 2 * stripe + k
                    pb = 64 * k
                    eng = nc.sync if k == 0 else nc.scalar
                    orr = out_ext[s].rearrange("c (n r) w -> n c r w", r=RC)
                    eng.dma_start(orr, ot[pb : pb + 64])
    nc.finalize()
    return nc


_nc_cache = None


def _get_nc():
    global _nc_cache
    if _nc_cache is None:
        _nc_cache = build_nc()
    return _nc_cache


def _run(x, sigma, trace=False):
    x = np.ascontiguousarray(x).astype(np.float16)
    sigma = np.ascontiguousarray(sigma).astype(np.float16)
    nc = _get_nc()
    in_maps = [
        {"x": x[S * i : S * (i + 1)], "sigma": sigma[S * i : S * (i + 1)]}
        for i in range(N_CORES)
    ]
    res = run_bass_kernel_spmd(nc, in_maps, list(range(N_CORES)), trace=trace)
    out = np.concatenate([res.results[i]["out"] for i in range(N_CORES)], axis=0)
    return out.astype(np.float32, copy=False), res


def kernel(x, sigma):
    out, _ = _run(x, sigma)
    return out


# revision 21
# speedup vs baseline: 1.2007x; 1.2007x over previous
import sys

import numpy as np

sys.path.insert(0, "/opt/trn_rl_repo")

from concourse import bacc, bass, tile  # noqa: E402,F401
from concourse import mybir  # noqa: E402
from concourse.bass import broadcast_tensor_aps  # noqa: E402
from concourse.bass_utils import run_bass_kernel_spmd  # noqa: E402

N_CORES = 8
S = 8  # samples per core
C = 3
T = 9
H = W = 256
RC = 4  # rows per chunk (one SBUF partition holds one chunk)
NCH = H // RC  # 64 chunks per sample
RP = RC + 2  # row slots incl top/bottom halo
WP = W + 2  # col slots incl left/right reflect pad
F32 = mybir.dt.float32
F16 = mybir.dt.float16
NPROD = 4  # product ring depth


def build_nc():
    nc = bacc.Bacc()
    x_ext = nc.declare_dram_parameter("x", [S, C, H, W], F16, isOutput=False)
    sg_ext = nc.declare_dram_parameter("sigma", [S, T, H, W], F16, isOutput=False)
    out_ext = nc.declare_dram_parameter("out", [S, C, H, W], F32, isOutput=True)

    with tile.TileContext(nc) as tc:
        with tc.tile_pool(name="p", bufs=2) as pool:
            for stripe in range(S // 2):
                xt = pool.tile([128, C, RP, WP], F16)
                st = pool.tile([128, T, RC, W], F16)
                prods = [
                    pool.tile([128, C, RC, W], F16, name=f"prod{j}")
                    for j in range(NPROD)
                ]
                acc = pool.tile([128, C, RC, W], F16)
                den16 = pool.tile([128, RC, W], F16)
                den = pool.tile([128, 1, RC, W], F32)
                inv = pool.tile([128, 1, RC, W], F32)
                ot = pool.tile([128, C, RC, W], F32)

                for k in range(2):
                    s = 2 * stripe + k
                    pb = 64 * k
                    # disjoint partition halves -> run the two samples' DMAs
                    # on separate engine queues
                    eng = nc.sync if k == 0 else nc.scalar
                    xr = x_ext[s].rearrange("c (n r) w -> n c r w", r=RC)
                    sr = sg_ext[s].rearrange("t (n r) w -> n t r w", r=RC)
                    # main rows -> slots 1..RC, image cols -> slots 1..W
                    # (DMA APs are limited to 3 dims -> one DMA per channel)
                    for c in range(C):
                        eng.dma_start(
                            xt[pb : pb + 64, c, 1 : 1 + RC, 1 : 1 + W], xr[:, c]
                        )
                    # top halo row: chunks 1..63 read prev chunk row 3
                    eng.dma_start(
                        xt[pb + 1 : pb + 64, :, 0, 1 : 1 + W], xr[0:63, :, 3, :]
                    )
                    # chunk 0 top halo: reflect row 1
                    eng.dma_start(xt[pb : pb + 1, :, 0, 1 : 1 + W], xr[0:1, :, 1, :])
                    # bottom halo row: chunks 0..62 read next chunk row 0
                    eng.dma_start(xt[pb : pb + 63, :, 5, 1 : 1 + W], xr[1:64, :, 0, :])
                    # chunk 63 bottom halo: reflect row 254 (= chunk 63 row 2)
                    eng.dma_start(
                        xt[pb + 63 : pb + 64, :, 5, 1 : 1 + W], xr[63:64, :, 2, :]
                    )
                    eng.dma_start(st[pb : pb + 64], sr)

                # column reflect pads: slot 0 <- image col 1 (slot 2),
                # slot WP-1 <- image col W-2 (slot WP-3)
                nc.scalar.copy(xt[:, :, :, 0:1], xt[:, :, :, 2:3])
                nc.scalar.copy(xt[:, :, :, WP - 1 : WP], xt[:, :, :, WP - 3 : WP - 2])

                # All compute on DVE: gpsimd touching recycled pool buffers
                # faults HW (NRT_EXEC_UNIT_UNRECOVERABLE 101). fp16 keeps
                # DVE in 2x_1p perf mode.
                with nc.allow_low_precision(reason="fp16 kernel"):
                    for t in range(T):
                        di, dj = t // 3, t % 3
                        xs = xt[:, :, di : di + RC, dj : dj + W]
                        sg = st[:, t : t + 1]
                        a, b = broadcast_tensor_aps(xs, sg)
                        nc.vector.tensor_mul(prods[t % NPROD][:], a, b)
                        if t == 1:
                            nc.vector.tensor_add(acc[:], prods[0][:], prods[1][:])
                        elif t > 1:
                            nc.vector.tensor_add(
                                acc[:], acc[:], prods[t % NPROD][:]
                            )

                    # DVE: denominator = sum sigma over taps (fp16 chain,
                    # fp32 tail), then reciprocal
                    nc.vector.tensor_add(den16[:], st[:, 0], st[:, 1])
                    for t in range(2, T - 1):
                        nc.vector.tensor_add(den16[:], den16[:], st[:, t])
                nc.vector.tensor_add(den[:, 0], den16[:], st[:, T - 1])
                nc.vector.reciprocal(inv[:], den[:])

                # normalize (fp32 out)
                a, b = broadcast_tensor_aps(acc[:], inv[:])
                nc.vector.tensor_mul(ot[:], a, b)

                for k in range(2):
                    s = 2 * stripe + k
                    pb = 64 * k
                    eng = nc.sync if k == 0 else nc.scalar
                    orr = out_ext[s].rearrange("c (n r) w -> n c r w", r=RC)
                    eng.dma_start(orr, ot[pb : pb + 64])
    nc.finalize()
    return nc


_nc_cache = None


def _get_nc():
    global _nc_cache
    if _nc_cache is None:
        _nc_cache = build_nc()
    return _nc_cache


def _run(x, sigma, trace=False):
    x = np.ascontiguousarray(x).astype(np.float16)
    sigma = np.ascontiguousarray(sigma).astype(np.float16)
    nc = _get_nc()
    in_maps = [
        {"x": x[S * i : S * (i + 1)], "sigma": sigma[S * i : S * (i + 1)]}
        for i in range(N_CORES)
    ]
    res = run_bass_kernel_spmd(nc, in_maps, list(range(N_CORES)), trace=trace)
    out = np.concatenate([res.results[i]["out"] for i in range(N_CORES)], axis=0)
    return out.astype(np.float32, copy=False), res


def kernel(x, sigma):
    out, _ = _run(x, sigma)
    return out


# revision 22
# speedup vs baseline: 1.2263x; 1.0213x over previous
import sys

import numpy as np

sys.path.insert(0, "/opt/trn_rl_repo")

from concourse import bacc, bass, tile  # noqa: E402,F401
from concourse import mybir  # noqa: E402
from concourse.bass import broadcast_tensor_aps  # noqa: E402
from concourse.bass_utils import run_bass_kernel_spmd  # noqa: E402

N_CORES = 8
S = 8  # samples per core
C = 3
T = 9
H = W = 256
RC = 4  # rows per chunk (one SBUF partition holds one chunk)
NCH = H // RC  # 64 chunks per sample
RP = RC + 2  # row slots incl top/bottom halo
WP = W + 2  # col slots incl left/right reflect pad
F32 = mybir.dt.float32
F16 = mybir.dt.float16
NPROD = 4  # product ring depth


def build_nc():
    nc = bacc.Bacc()
    x_ext = nc.declare_dram_parameter("x", [S, C, H, W], F16, isOutput=False)
    sg_ext = nc.declare_dram_parameter("sigma", [S, T, H, W], F16, isOutput=False)
    out_ext = nc.declare_dram_parameter("out", [S, C, H, W], F32, isOutput=True)

    with tile.TileContext(nc) as tc:
        with tc.tile_pool(name="p", bufs=2) as pool:
            for stripe in range(S // 2):
                xt = pool.tile([128, C, RP, WP], F16)
                st = pool.tile([128, T, RC, W], F16)
                prods = [
                    pool.tile([128, C, RC, W], F16, name=f"prod{j}")
                    for j in range(NPROD)
                ]
                acc = pool.tile([128, C, RC, W], F16)
                den16 = pool.tile([128, RC, W], F16)
                den = pool.tile([128, 1, RC, W], F32)
                inv = pool.tile([128, 1, RC, W], F32)
                ot = pool.tile([128, C, RC, W], F32)

                for k in range(2):
                    s = 2 * stripe + k
                    pb = 64 * k
                    # disjoint partition halves -> run the two samples' DMAs
                    # on separate engine queues
                    eng = nc.sync if k == 0 else nc.scalar
                    xr = x_ext[s].rearrange("c (n r) w -> n c r w", r=RC)
                    sr = sg_ext[s].rearrange("t (n r) w -> n t r w", r=RC)
                    # sigma first: den chain can start before x lands
                    eng.dma_start(st[pb : pb + 64], sr)
                    # main rows -> slots 1..RC, image cols -> slots 1..W
                    # (DMA APs are limited to 3 dims -> one DMA per channel)
                    for c in range(C):
                        eng.dma_start(
                            xt[pb : pb + 64, c, 1 : 1 + RC, 1 : 1 + W], xr[:, c]
                        )
                    # top halo row: chunks 1..63 read prev chunk row 3
                    eng.dma_start(
                        xt[pb + 1 : pb + 64, :, 0, 1 : 1 + W], xr[0:63, :, 3, :]
                    )
                    # chunk 0 top halo: reflect row 1
                    eng.dma_start(xt[pb : pb + 1, :, 0, 1 : 1 + W], xr[0:1, :, 1, :])
                    # bottom halo row: chunks 0..62 read next chunk row 0
                    eng.dma_start(xt[pb : pb + 63, :, 5, 1 : 1 + W], xr[1:64, :, 0, :])
                    # chunk 63 bottom halo: reflect row 254 (= chunk 63 row 2)
                    eng.dma_start(
                        xt[pb + 63 : pb + 64, :, 5, 1 : 1 + W], xr[63:64, :, 2, :]
                    )

                # column reflect pads: slot 0 <- image col 1 (slot 2),
                # slot WP-1 <- image col W-2 (slot WP-3)
                nc.scalar.copy(xt[:, :, :, 0:1], xt[:, :, :, 2:3])
                nc.scalar.copy(xt[:, :, :, WP - 1 : WP], xt[:, :, :, WP - 3 : WP - 2])

                # All compute on DVE: gpsimd touching recycled pool buffers
                # faults HW (NRT_EXEC_UNIT_UNRECOVERABLE 101). fp16 keeps
                # DVE in 2x_1p perf mode.
                # den chain first: only needs sigma, which lands before x
                with nc.allow_low_precision(reason="fp16 kernel"):
                    nc.vector.tensor_add(den16[:], st[:, 0], st[:, 1])
                    for t in range(2, T - 1):
                        nc.vector.tensor_add(den16[:], den16[:], st[:, t])
                nc.vector.tensor_add(den[:, 0], den16[:], st[:, T - 1])
                # ~5x faster than reciprocal(); ~18 correct bits >> fp16
                # noise floor, den in [0.8, 9] so no edge cases
                nc.vector.reciprocal_approx_fast(inv[:, 0], den[:, 0])
                with nc.allow_low_precision(reason="fp16 kernel"):
                    for t in range(T):
                        di, dj = t // 3, t % 3
                        xs = xt[:, :, di : di + RC, dj : dj + W]
                        sg = st[:, t : t + 1]
                        a, b = broadcast_tensor_aps(xs, sg)
                        nc.vector.tensor_mul(prods[t % NPROD][:], a, b)
                        if t == 1:
                            nc.vector.tensor_add(acc[:], prods[0][:], prods[1][:])
                        elif t > 1:
                            nc.vector.tensor_add(
                                acc[:], acc[:], prods[t % NPROD][:]
                            )

                # normalize (fp32 out)
                a, b = broadcast_tensor_aps(acc[:], inv[:])
                nc.vector.tensor_mul(ot[:], a, b)

                for k in range(2):
                    s = 2 * stripe + k
                    pb = 64 * k
                    eng = nc.sync if k == 0 else nc.scalar
                    orr = out_ext[s].rearrange("c (n r) w -> n c r w", r=RC)
                    eng.dma_start(orr, ot[pb : pb + 64])
    nc.finalize()
    return nc


_nc_cache = None


def _get_nc():
    global _nc_cache
    if _nc_cache is None:
        _nc_cache = build_nc()
    return _nc_cache


def _run(x, sigma, trace=False):
    x = np.ascontiguousarray(x).astype(np.float16)
    sigma = np.ascontiguousarray(sigma).astype(np.float16)
    nc = _get_nc()
    in_maps = [
        {"x": x[S * i : S * (i + 1)], "sigma": sigma[S * i : S * (i + 1)]}
        for i in range(N_CORES)
    ]
    res = run_bass_kernel_spmd(nc, in_maps, list(range(N_CORES)), trace=trace)
    out = np.concatenate([res.results[i]["out"] for i in range(N_CORES)], axis=0)
    return out.astype(np.float32, copy=False), res


def kernel(x, sigma):
    out, _ = _run(x, sigma)
    return out


# revision 23
# speedup vs baseline: 1.2371x; 1.0088x over previous
import sys

import numpy as np

sys.path.insert(0, "/opt/trn_rl_repo")

from concourse import bacc, bass, tile  # noqa: E402,F401
from concourse import mybir  # noqa: E402
from concourse.bass import broadcast_tensor_aps  # noqa: E402
from concourse.bass_utils import run_bass_kernel_spmd  # noqa: E402

N_CORES = 8
S = 8  # samples per core
C = 3
T = 9
H = W = 256
RC = 4  # rows per chunk (one SBUF partition holds one chunk)
NCH = H // RC  # 64 chunks per sample
RP = RC + 2  # row slots incl top/bottom halo
WP = W + 2  # col slots incl left/right reflect pad
F32 = mybir.dt.float32
F16 = mybir.dt.float16
NPROD = 4  # product ring depth


def build_nc():
    nc = bacc.Bacc()
    x_ext = nc.declare_dram_parameter("x", [S, C, H, W], F16, isOutput=False)
    sg_ext = nc.declare_dram_parameter("sigma", [S, T, H, W], F16, isOutput=False)
    out_ext = nc.declare_dram_parameter("out", [S, C, H, W], F32, isOutput=True)

    with tile.TileContext(nc) as tc:
        with tc.tile_pool(name="p", bufs=2) as pool:
            for stripe in range(S // 2):
                xt = pool.tile([128, C, RP, WP], F16)
                st = pool.tile([128, T, RC, W], F16)
                prods = [
                    pool.tile([128, C, RC, W], F16, name=f"prod{j}")
                    for j in range(NPROD)
                ]
                acc = pool.tile([128, C, RC, W], F16)
                den16 = pool.tile([128, RC, W], F16)
                den = pool.tile([128, 1, RC, W], F32)
                inv = pool.tile([128, 1, RC, W], F32)
                ot = pool.tile([128, C, RC, W], F32)

                for k in range(2):
                    s = 2 * stripe + k
                    pb = 64 * k
                    # disjoint partition halves -> run the two samples' DMAs
                    # on separate engine queues
                    eng = nc.sync if k == 0 else nc.scalar
                    xr = x_ext[s].rearrange("c (n r) w -> n c r w", r=RC)
                    sr = sg_ext[s].rearrange("t (n r) w -> n t r w", r=RC)
                    # main rows -> slots 1..RC, image cols -> slots 1..W
                    # (DMA APs are limited to 3 dims -> one DMA per channel)
                    for c in range(C):
                        eng.dma_start(
                            xt[pb : pb + 64, c, 1 : 1 + RC, 1 : 1 + W], xr[:, c]
                        )
                    # top halo row: chunks 1..63 read prev chunk row 3
                    eng.dma_start(
                        xt[pb + 1 : pb + 64, :, 0, 1 : 1 + W], xr[0:63, :, 3, :]
                    )
                    # chunk 0 top halo: reflect row 1
                    eng.dma_start(xt[pb : pb + 1, :, 0, 1 : 1 + W], xr[0:1, :, 1, :])
                    # bottom halo row: chunks 0..62 read next chunk row 0
                    eng.dma_start(xt[pb : pb + 63, :, 5, 1 : 1 + W], xr[1:64, :, 0, :])
                    # chunk 63 bottom halo: reflect row 254 (= chunk 63 row 2)
                    eng.dma_start(
                        xt[pb + 63 : pb + 64, :, 5, 1 : 1 + W], xr[63:64, :, 2, :]
                    )
                    # sigma streamed per tap after x: stripe-0 muls can start
                    # as soon as x + tap 0 land instead of after all of sigma
                    for t in range(T):
                        eng.dma_start(st[pb : pb + 64, t], sr[:, t])

                # column reflect pads: slot 0 <- image col 1 (slot 2),
                # slot WP-1 <- image col W-2 (slot WP-3)
                nc.scalar.copy(xt[:, :, :, 0:1], xt[:, :, :, 2:3])
                nc.scalar.copy(xt[:, :, :, WP - 1 : WP], xt[:, :, :, WP - 3 : WP - 2])

                # All compute on DVE: gpsimd touching recycled pool buffers
                # faults HW (NRT_EXEC_UNIT_UNRECOVERABLE 101). fp16 keeps
                # DVE in 2x_1p perf mode.
                with nc.allow_low_precision(reason="fp16 kernel"):
                    for t in range(T):
                        di, dj = t // 3, t % 3
                        xs = xt[:, :, di : di + RC, dj : dj + W]
                        sg = st[:, t : t + 1]
                        a, b = broadcast_tensor_aps(xs, sg)
                        nc.vector.tensor_mul(prods[t % NPROD][:], a, b)
                        if t == 1:
                            nc.vector.tensor_add(acc[:], prods[0][:], prods[1][:])
                        elif t > 1:
                            nc.vector.tensor_add(
                                acc[:], acc[:], prods[t % NPROD][:]
                            )

                    nc.vector.tensor_add(den16[:], st[:, 0], st[:, 1])
                    for t in range(2, T - 1):
                        nc.vector.tensor_add(den16[:], den16[:], st[:, t])
                nc.vector.tensor_add(den[:, 0], den16[:], st[:, T - 1])
                # ~5x faster than reciprocal(); ~18 correct bits >> fp16
                # noise floor, den in [0.8, 9] so no edge cases
                nc.vector.reciprocal_approx_fast(inv[:, 0], den[:, 0])

                # normalize + store per channel: out DMA of channel c starts
                # while channel c+1 is still normalizing (shrinks the tail)
                for c in range(C):
                    nc.vector.tensor_mul(ot[:, c], acc[:, c], inv[:, 0])
                    for k in range(2):
                        s = 2 * stripe + k
                        pb = 64 * k
                        eng = nc.sync if k == 0 else nc.scalar
                        orr = out_ext[s].rearrange("c (n r) w -> n c r w", r=RC)
                        eng.dma_start(orr[:, c], ot[pb : pb + 64, c])

    nc.finalize()
    return nc


_nc_cache = None


def _get_nc():
    global _nc_cache
    if _nc_cache is None:
        _nc_cache = build_nc()
    return _nc_cache


def _run(x, sigma, trace=False):
    x = np.ascontiguousarray(x).astype(np.float16)
    sigma = np.ascontiguousarray(sigma).astype(np.float16)
    nc = _get_nc()
    in_maps = [
        {"x": x[S * i : S * (i + 1)], "sigma": sigma[S * i : S * (i + 1)]}
        for i in range(N_CORES)
    ]
    res = run_bass_kernel_spmd(nc, in_maps, list(range(N_CORES)), trace=trace)
    out = np.concatenate([res.results[i]["out"] for i in range(N_CORES)], axis=0)
    return out.astype(np.float32, copy=False), res


def kernel(x, sigma):
    out, _ = _run(x, sigma)
    return out
